# revision 18
# baseline (speedup 1.0000x reference)
"""Trainium2 Bass kernel for nn_AttentionBlock_90537910600269.

Computes, per position (B=64, L=16384, C=64):
    xa = xatt[b, l, :] . W_act + b_act          (scalar)
    xs = xsaut[b, l, :] . W_saut + b_saut       (scalar)
    h  = leaky_relu(xa + xs, 0.3)
    g  = sigmoid(h * W2 + b2)
    out[b, l, 0] = g * xs

Sharding: pure data parallel, batch dim split across 8 NeuronCores
(8 batches per core). Params are tiny and replicated (pre-broadcast on
host to [128, C] / [128, 1] so they can be used as per-partition
operands on-chip).

Per-core layout: the 8-batch shard is flattened to [131072, 64] rows
and rearranged "(p r) c -> p (r c)" with p=128, so each SBUF partition
owns 1024 consecutive rows (256 KB contiguous in HBM). Tiles of T=128
rows/partition give fully-contiguous 32 KiB-per-partition DMA
descriptors (4 MiB per dma_start, spanning all 128 partitions / all 16
DMA ports).

Engine split per tile (FD = T*C = 8192):
    DVE    : x * W broadcast multiply (in-place) x2, grouped reduce
             over C (axis=X) x2
    GPSIMD : h = xa + xs + bias, leaky-relu via max(h, 0.3h),
             final (xs + b_saut) * g
    ACT    : sigmoid(W2 * h + b2)
    DMA    : HWDGE (nc.sync) only, so no SWDGE/GPSIMD interaction
"""

import numpy as np

B, L, C = 64, 16384, 64
N_CORES = 8
BPC = B // N_CORES          # batches per core
ROWS = BPC * L              # rows per core = 131072
P = 128                     # SBUF partitions
RPP = ROWS // P             # rows per partition = 1024
T = 128                     # rows per partition per tile
NT = RPP // T               # tiles per core
import os as _os

GD = int(_os.environ.get("KGD", "68"))    # rows/tile (per stream) multiplied on DVE
BUFS = int(_os.environ.get("KBUFS", "3"))  # big-tile buffers (DMA prefetch depth)
NEG_SLOPE = 0.3

_PROGRAMS = {}


def _build_program(repeat=1, mode="full"):
    """Build the per-core Bass program.

    repeat > 1 re-runs the whole pass over the same DRAM inputs multiple
    times inside one NEFF — used only for steady-state benchmarking
    (the per-call dispatch overhead is amortized / cancelled by
    differencing two repeat counts).

    mode: "full" | "dma" (no compute) | "mul" (muls only) |
          "red" (reduces only) — bottleneck isolation probes.
    """
    import concourse.mybir as mybir
    from concourse import bacc
    from concourse.tile import TileContext

    f32 = mybir.dt.float32
    Alu = mybir.AluOpType

    nc = bacc.Bacc(
        "TRN2",
        target_bir_lowering=False,
        debug=False,
        num_devices=N_CORES,
    )

    x_att = nc.dram_tensor("xatt", [ROWS, C], f32, kind="ExternalInput")
    x_sau = nc.dram_tensor("xsaut", [ROWS, C], f32, kind="ExternalInput")
    w_a = nc.dram_tensor("wa", [P, C], f32, kind="ExternalInput")
    w_s = nc.dram_tensor("ws", [P, C], f32, kind="ExternalInput")
    # columns: 0 = b_act + b_saut, 1 = W2, 2 = b2, 3 = b_saut
    sc = nc.dram_tensor("sc", [P, 4], f32, kind="ExternalInput")
    y = nc.dram_tensor("y", [ROWS], f32, kind="ExternalOutput")

    xa_r = x_att[:].rearrange("(p r) c -> p (r c)", p=P)
    xs_r = x_sau[:].rearrange("(p r) c -> p (r c)", p=P)
    y_r = y[:].rearrange("(p r) -> p r", p=P)

    TC = T * C

    with TileContext(nc) as tc:
        with (
            tc.tile_pool(name="wpool", bufs=1) as wpool,
            tc.tile_pool(name="big", bufs=BUFS) as big,
            tc.tile_pool(name="small", bufs=4) as small,
        ):
            # weight pair tile: [P, 2, C] with wa in slot 0, ws in slot 1
            wp_t = wpool.tile([P, 2 * C], f32, tag="wp")
            nc.sync.dma_start(out=wp_t[:, 0:C], in_=w_a[:])
            nc.sync.dma_start(out=wp_t[:, C:2 * C], in_=w_s[:])
            sc_t = wpool.tile([P, 4], f32, tag="sc")
            nc.sync.dma_start(out=sc_t[:], in_=sc[:])
            bias_h = sc_t[:, 0:1]
            w2_ap = sc_t[:, 1:2]
            b2_ap = sc_t[:, 2:3]
            bs_ap = sc_t[:, 3:4]

            wp3 = wp_t[:].rearrange("p (s c) -> p s c", c=C)

            def wp_b(rows):
                # [P, 2, rows, C] broadcast of the weight pair
                return wp3[:, :, None, :].broadcast_to([P, 2, rows, C])

            for _rep in range(repeat):
              for t in range(NT):
                # one combined tile: xatt chunk | xsaut chunk
                x_t = big.tile([P, 2 * TC], f32, tag="x")
                nc.sync.dma_start(
                    out=x_t[:, 0:TC], in_=xa_r[:, t * TC:(t + 1) * TC]
                )
                nc.sync.dma_start(
                    out=x_t[:, TC:2 * TC], in_=xs_r[:, t * TC:(t + 1) * TC]
                )
                x4 = x_t[:].rearrange("p (s r c) -> p s r c", s=2, c=C)

                if mode in ("full", "mul"):
                    # z = x * W (broadcast along rows), in place; DVE takes
                    # rows [0:GD) of both streams, GPSIMD rows [GD:T).
                    # (GPSIMD TT and DVE TT/reduce use disjoint SBUF ports.)
                    if GD < T:
                        nc.gpsimd.tensor_mul(
                            out=x4[:, :, GD:T, :], in0=x4[:, :, GD:T, :],
                            in1=wp_b(T - GD),
                        )
                    if GD > 0:
                        nc.vector.tensor_mul(
                            out=x4[:, :, 0:GD, :], in0=x4[:, :, 0:GD, :],
                            in1=wp_b(GD),
                        )

                s2 = small.tile([P, 2 * T], f32, tag="s2")
                if mode in ("full", "red"):
                    # grouped reduce over C: [P, 2, T, C] -> [P, 2T]
                    nc.vector.reduce_sum(
                        out=s2[:], in_=x4, axis=mybir.AxisListType.X
                    )
                else:
                    nc.vector.tensor_copy(out=s2[:], in_=x_t[:, 0:2 * T])
                xa_s = s2[:, 0:T]
                xs_s = s2[:, T:2 * T]

                res = small.tile([P, T], f32, tag="res")
                if mode in ("full",):
                    # h = (xa + (b_act + b_saut)) + xs
                    h = small.tile([P, T], f32, tag="h")
                    nc.vector.scalar_tensor_tensor(
                        out=h[:], in0=xa_s, scalar=bias_h, in1=xs_s,
                        op0=Alu.add, op1=Alu.add,
                    )
                    # leaky relu: max(h, 0.3 * h)  (ACT's Lrelu hardwires
                    # slope 0.01 and ignores alpha — measured on HW)
                    lk = small.tile([P, T], f32, tag="lk")
                    nc.vector.scalar_tensor_tensor(
                        out=lk[:], in0=h[:], scalar=float(NEG_SLOPE), in1=h[:],
                        op0=Alu.mult, op1=Alu.max,
                    )
                    # g = sigmoid(W2 * lk + b2)
                    g = small.tile([P, T], f32, tag="g")
                    nc.scalar.activation(
                        out=g[:], in_=lk[:],
                        func=mybir.ActivationFunctionType.Sigmoid,
                        bias=b2_ap, scale=w2_ap,
                    )
                    # out = (xs + b_saut) * g
                    nc.vector.scalar_tensor_tensor(
                        out=res[:], in0=xs_s, scalar=bs_ap, in1=g[:],
                        op0=Alu.add, op1=Alu.mult,
                    )
                else:
                    nc.vector.tensor_add(out=res[:], in0=xa_s, in1=xs_s)
                nc.sync.dma_start(out=y_r[:, t * T:(t + 1) * T], in_=res[:])

    nc.finalize()
    return nc


def _get_program(repeat=1, mode="full"):
    key = (repeat, mode)
    if key not in _PROGRAMS:
        _PROGRAMS[key] = _build_program(repeat, mode)
    return _PROGRAMS[key]


def _make_in_maps(xatt, xsaut, W_act, b_act, W_saut, b_saut, W2, b2,
                  w_scale=1.0):
    xatt = np.asarray(xatt, np.float32)
    xsaut = np.asarray(xsaut, np.float32)
    wa = np.ascontiguousarray(
        np.broadcast_to(
            np.asarray(W_act, np.float32).reshape(1, C) * w_scale, (P, C)
        )
    )
    ws = np.ascontiguousarray(
        np.broadcast_to(
            np.asarray(W_saut, np.float32).reshape(1, C) * w_scale, (P, C)
        )
    )
    b_act_v = float(np.asarray(b_act).reshape(-1)[0])
    b_saut_v = float(np.asarray(b_saut).reshape(-1)[0])
    w2_v = float(np.asarray(W2).reshape(-1)[0])
    b2_v = float(np.asarray(b2).reshape(-1)[0])
    sc = np.ascontiguousarray(
        np.broadcast_to(
            np.array([[b_act_v + b_saut_v, w2_v, b2_v, b_saut_v]], np.float32),
            (P, 4),
        )
    )
    in_maps = []
    for k in range(N_CORES):
        xa_k = np.ascontiguousarray(
            xatt[k * BPC:(k + 1) * BPC].reshape(ROWS, C)
        )
        xs_k = np.ascontiguousarray(
            xsaut[k * BPC:(k + 1) * BPC].reshape(ROWS, C)
        )
        in_maps.append(
            {"xatt": xa_k, "xsaut": xs_k, "wa": wa, "ws": ws, "sc": sc}
        )
    return in_maps


def _run(inputs, trace=False, trace_kwargs=None):
    """Returns (full_output, BassKernelResults)."""
    from concourse import bass_utils

    nc = _get_program()
    in_maps = _make_in_maps(**inputs)
    kw = {}
    if trace:
        kw["trace"] = True
        if trace_kwargs:
            kw["trace_kwargs"] = trace_kwargs
    res = bass_utils.run_bass_kernel_spmd(
        nc, in_maps, core_ids=list(range(N_CORES)), **kw
    )
    parts = [
        np.asarray(r["y"], np.float32).reshape(BPC, L, 1) for r in res.results
    ]
    out = np.concatenate(parts, axis=0)
    return out, res


def kernel(**inputs):
    out, _ = _run(inputs, trace=False)
    return out


# revision 19
# speedup vs baseline: 4.0434x; 4.0434x over previous
"""Trainium2 Bass kernel for nn_AttentionBlock_90537910600269.

Computes, per position (B=64, L=16384, C=64):
    xa = xatt[b, l, :] . W_act + b_act          (scalar)
    xs = xsaut[b, l, :] . W_saut + b_saut       (scalar)
    h  = leaky_relu(xa + xs, 0.3)
    g  = sigmoid(h * W2 + b2)
    out[b, l, 0] = g * xs

Sharding: pure data parallel, batch dim split across 8 NeuronCores
(8 batches per core). Params are tiny and replicated (pre-broadcast on
host to [128, C] / [128, 1] so they can be used as per-partition
operands on-chip).

Per-core layout: the 8-batch shard is flattened to [131072, 64] rows
and rearranged "(p r) c -> p (r c)" with p=128, so each SBUF partition
owns 1024 consecutive rows (256 KB contiguous in HBM). Per tile, both
streams land in ONE combined SBUF tile [128, 2*T*C] via two 4 MiB
dma_starts with fully-contiguous 32 KiB-per-partition descriptors
(all 128 partitions -> all 16 DMA ports).

Engine split per tile (x4 view = [P, 2 streams, T=128 rows, C]):
    DVE    : in-place x*W broadcast multiply on rows [0:GD) of both
             streams (1 op), grouped reduce over C axis=X for both
             streams (1 op, [P,2,T,C] -> [P,2T]), h = xa+xs+bias (STT),
             leaky = max(h, 0.3h) (STT), out = (xs+b_saut)*g (STT)
    GPSIMD : in-place x*W multiply on rows [GD:T) of both streams
             (1 op; GPSIMD TT never contends with DVE's 1-port
             TT/reduce ops for SBUF)
    ACT    : g = sigmoid(W2 * leaky + b2)
    DMA    : HWDGE (nc.sync) only, so no SWDGE/GPSIMD interaction

GD=68 balances DVE (~27 us/tile) against GPSIMD's ~2.9 ns/elem
effective TT rate (~22 us/tile), both under the ~25-27 us/tile DMA
shadow. Measured ~195-215 us/core vs 189 us HBM roofline (67.6 MB @
358 GB/s). All-DVE baseline was 316 us; pool/PE-transpose/ACT-accum
reduction alternatives are unusable on this toolchain (pool fails ISA
check, ACT Lrelu ignores alpha, PE transpose is latency-bound).
"""

import numpy as np

B, L, C = 64, 16384, 64
N_CORES = 8
BPC = B // N_CORES          # batches per core
ROWS = BPC * L              # rows per core = 131072
P = 128                     # SBUF partitions
RPP = ROWS // P             # rows per partition = 1024
T = 128                     # rows per partition per tile
NT = RPP // T               # tiles per core
import os as _os

GD = int(_os.environ.get("KGD", "68"))    # rows/tile (per stream) multiplied on DVE
BUFS = int(_os.environ.get("KBUFS", "3"))  # big-tile buffers (DMA prefetch depth)
NEG_SLOPE = 0.3

_PROGRAMS = {}


def _build_program(repeat=1, mode="full"):
    """Build the per-core Bass program.

    repeat > 1 re-runs the whole pass over the same DRAM inputs multiple
    times inside one NEFF — used only for steady-state benchmarking
    (the per-call dispatch overhead is amortized / cancelled by
    differencing two repeat counts).

    mode: "full" | "dma" (no compute) | "mul" (muls only) |
          "red" (reduces only) — bottleneck isolation probes.
    """
    import concourse.mybir as mybir
    from concourse import bacc
    from concourse.tile import TileContext

    f32 = mybir.dt.float32
    Alu = mybir.AluOpType

    nc = bacc.Bacc(
        "TRN2",
        target_bir_lowering=False,
        debug=False,
        num_devices=N_CORES,
    )

    x_att = nc.dram_tensor("xatt", [ROWS, C], f32, kind="ExternalInput")
    x_sau = nc.dram_tensor("xsaut", [ROWS, C], f32, kind="ExternalInput")
    w_a = nc.dram_tensor("wa", [P, C], f32, kind="ExternalInput")
    w_s = nc.dram_tensor("ws", [P, C], f32, kind="ExternalInput")
    # columns: 0 = b_act + b_saut, 1 = W2, 2 = b2, 3 = b_saut
    sc = nc.dram_tensor("sc", [P, 4], f32, kind="ExternalInput")
    y = nc.dram_tensor("y", [ROWS], f32, kind="ExternalOutput")

    xa_r = x_att[:].rearrange("(p r) c -> p (r c)", p=P)
    xs_r = x_sau[:].rearrange("(p r) c -> p (r c)", p=P)
    y_r = y[:].rearrange("(p r) -> p r", p=P)

    TC = T * C

    with TileContext(nc) as tc:
        with (
            tc.tile_pool(name="wpool", bufs=1) as wpool,
            tc.tile_pool(name="big", bufs=BUFS) as big,
            tc.tile_pool(name="small", bufs=4) as small,
        ):
            # weight pair tile: [P, 2, C] with wa in slot 0, ws in slot 1
            wp_t = wpool.tile([P, 2 * C], f32, tag="wp")
            nc.sync.dma_start(out=wp_t[:, 0:C], in_=w_a[:])
            nc.sync.dma_start(out=wp_t[:, C:2 * C], in_=w_s[:])
            sc_t = wpool.tile([P, 4], f32, tag="sc")
            nc.sync.dma_start(out=sc_t[:], in_=sc[:])
            bias_h = sc_t[:, 0:1]
            w2_ap = sc_t[:, 1:2]
            b2_ap = sc_t[:, 2:3]
            bs_ap = sc_t[:, 3:4]

            wp3 = wp_t[:].rearrange("p (s c) -> p s c", c=C)

            def wp_b(rows):
                # [P, 2, rows, C] broadcast of the weight pair
                return wp3[:, :, None, :].broadcast_to([P, 2, rows, C])

            for _rep in range(repeat):
              for t in range(NT):
                # one combined tile: xatt chunk | xsaut chunk
                x_t = big.tile([P, 2 * TC], f32, tag="x")
                nc.sync.dma_start(
                    out=x_t[:, 0:TC], in_=xa_r[:, t * TC:(t + 1) * TC]
                )
                nc.sync.dma_start(
                    out=x_t[:, TC:2 * TC], in_=xs_r[:, t * TC:(t + 1) * TC]
                )
                x4 = x_t[:].rearrange("p (s r c) -> p s r c", s=2, c=C)

                if mode in ("full", "mul"):
                    # z = x * W (broadcast along rows), in place; DVE takes
                    # rows [0:GD) of both streams, GPSIMD rows [GD:T).
                    # (GPSIMD TT and DVE TT/reduce use disjoint SBUF ports.)
                    if GD < T:
                        nc.gpsimd.tensor_mul(
                            out=x4[:, :, GD:T, :], in0=x4[:, :, GD:T, :],
                            in1=wp_b(T - GD),
                        )
                    if GD > 0:
                        nc.vector.tensor_mul(
                            out=x4[:, :, 0:GD, :], in0=x4[:, :, 0:GD, :],
                            in1=wp_b(GD),
                        )

                s2 = small.tile([P, 2 * T], f32, tag="s2")
                if mode in ("full", "red"):
                    # grouped reduce over C: [P, 2, T, C] -> [P, 2T]
                    nc.vector.reduce_sum(
                        out=s2[:], in_=x4, axis=mybir.AxisListType.X
                    )
                else:
                    nc.vector.tensor_copy(out=s2[:], in_=x_t[:, 0:2 * T])
                xa_s = s2[:, 0:T]
                xs_s = s2[:, T:2 * T]

                res = small.tile([P, T], f32, tag="res")
                if mode in ("full",):
                    # h = (xa + (b_act + b_saut)) + xs
                    h = small.tile([P, T], f32, tag="h")
                    nc.vector.scalar_tensor_tensor(
                        out=h[:], in0=xa_s, scalar=bias_h, in1=xs_s,
                        op0=Alu.add, op1=Alu.add,
                    )
                    # leaky relu: max(h, 0.3 * h)  (ACT's Lrelu hardwires
                    # slope 0.01 and ignores alpha — measured on HW)
                    lk = small.tile([P, T], f32, tag="lk")
                    nc.vector.scalar_tensor_tensor(
                        out=lk[:], in0=h[:], scalar=float(NEG_SLOPE), in1=h[:],
                        op0=Alu.mult, op1=Alu.max,
                    )
                    # g = sigmoid(W2 * lk + b2)
                    g = small.tile([P, T], f32, tag="g")
                    nc.scalar.activation(
                        out=g[:], in_=lk[:],
                        func=mybir.ActivationFunctionType.Sigmoid,
                        bias=b2_ap, scale=w2_ap,
                    )
                    # out = (xs + b_saut) * g
                    nc.vector.scalar_tensor_tensor(
                        out=res[:], in0=xs_s, scalar=bs_ap, in1=g[:],
                        op0=Alu.add, op1=Alu.mult,
                    )
                else:
                    nc.vector.tensor_add(out=res[:], in0=xa_s, in1=xs_s)
                nc.sync.dma_start(out=y_r[:, t * T:(t + 1) * T], in_=res[:])

    nc.finalize()
    return nc


def _get_program(repeat=1, mode="full"):
    key = (repeat, mode)
    if key not in _PROGRAMS:
        _PROGRAMS[key] = _build_program(repeat, mode)
    return _PROGRAMS[key]


def _make_in_maps(xatt, xsaut, W_act, b_act, W_saut, b_saut, W2, b2,
                  w_scale=1.0):
    xatt = np.asarray(xatt, np.float32)
    xsaut = np.asarray(xsaut, np.float32)
    wa = np.ascontiguousarray(
        np.broadcast_to(
            np.asarray(W_act, np.float32).reshape(1, C) * w_scale, (P, C)
        )
    )
    ws = np.ascontiguousarray(
        np.broadcast_to(
            np.asarray(W_saut, np.float32).reshape(1, C) * w_scale, (P, C)
        )
    )
    b_act_v = float(np.asarray(b_act).reshape(-1)[0])
    b_saut_v = float(np.asarray(b_saut).reshape(-1)[0])
    w2_v = float(np.asarray(W2).reshape(-1)[0])
    b2_v = float(np.asarray(b2).reshape(-1)[0])
    sc = np.ascontiguousarray(
        np.broadcast_to(
            np.array([[b_act_v + b_saut_v, w2_v, b2_v, b_saut_v]], np.float32),
            (P, 4),
        )
    )
    in_maps = []
    for k in range(N_CORES):
        xa_k = np.ascontiguousarray(
            xatt[k * BPC:(k + 1) * BPC].reshape(ROWS, C)
        )
        xs_k = np.ascontiguousarray(
            xsaut[k * BPC:(k + 1) * BPC].reshape(ROWS, C)
        )
        in_maps.append(
            {"xatt": xa_k, "xsaut": xs_k, "wa": wa, "ws": ws, "sc": sc}
        )
    return in_maps


def _run(inputs, trace=False, trace_kwargs=None):
    """Returns (full_output, BassKernelResults)."""
    from concourse import bass_utils

    nc = _get_program()
    in_maps = _make_in_maps(**inputs)
    kw = {}
    if trace:
        kw["trace"] = True
        if trace_kwargs:
            kw["trace_kwargs"] = trace_kwargs
    res = bass_utils.run_bass_kernel_spmd(
        nc, in_maps, core_ids=list(range(N_CORES)), **kw
    )
    parts = [
        np.asarray(r["y"], np.float32).reshape(BPC, L, 1) for r in res.results
    ]
    out = np.concatenate(parts, axis=0)
    return out, res


def kernel(**inputs):
    out, _ = _run(inputs, trace=False)
    return out
